# revision 4
# baseline (speedup 1.0000x reference)
"""GQA attention on 8 TRN2 NeuronCores.

Problem: x[4,2048,2048] -> q/kv proj -> causal GQA (16 q heads, 4 kv heads,
head_dim 128) -> c_proj.  f32 in/out.

Sharding: core = (batch b, head-group hg); b = core//2, hg = core%2.
Each core handles 1 batch, 8 query heads (hg*8..hg*8+7) and their 2 shared
KV heads, plus the c_proj partial over its 1024 features.  Host sums the two
c_proj partials per batch and adds the bias corrections.

On-device layout strategy (zero on-device transposes):
 - host passes xT = x[b].T (bf16, contiguous)
 - qT[d,i], kT[d,j] computed directly transposed:  lhsT=W chunk, rhs=xT chunk
 - v[j,d] natural:                                 lhsT=xT chunk, rhs=Wv
 - scores ST[j,i] = kT.T @ qT                      (128-wide j blocks)
 - ET = exp(scale*ST) on ScalarE, 0/1 causal mask multiply on diagonal blocks
 - outT[d,i] accumulates  lhsT=v block, rhs=ET
 - row sums l[i] via all-ones 128x128 lhsT matmul (broadcast rows), then
   DVE reciprocal + multiply to normalize
 - c_proj out[i,Dout] accumulates over heads:      lhsT=outT slice, rhs=Wc
"""

import os
import sys

import numpy as np

for _p in ("/opt/trn_rl_repo",):
    if _p not in sys.path and os.path.isdir(_p):
        sys.path.insert(0, _p)

import ml_dtypes  # noqa: E402
from contextlib import ExitStack  # noqa: E402

import concourse.bass as bass  # noqa: E402
import concourse.mybir as mybir  # noqa: E402
import concourse.tile as tile  # noqa: E402
from concourse import bacc  # noqa: E402
from concourse import bass_utils  # noqa: E402

BF16 = ml_dtypes.bfloat16

B, S, D = 4, 2048, 2048
N_HEAD, GROUP = 16, 4
HD = 128
H_LOC, KV_LOC = 8, 2          # per-core query heads / kv heads
SCALE = 1.0 / float(np.sqrt(HD))
NT = S // 512                  # i-tiles of 512
NCH = D // 128                 # contraction chunks
NJB = S // 128                 # j blocks

LAST_RESULTS = None            # BassKernelResults of the last run (for test.py)


def _build_program():
    nc = bacc.Bacc(None, target_bir_lowering=False)
    bf = mybir.dt.bfloat16
    f32 = mybir.dt.float32

    xT = nc.dram_tensor("xt", [D, S], bf, kind="ExternalInput")
    wq = nc.dram_tensor("wq", [D, H_LOC * HD], bf, kind="ExternalInput")
    wk = nc.dram_tensor("wk", [D, KV_LOC * HD], bf, kind="ExternalInput")
    wv = nc.dram_tensor("wv", [D, KV_LOC * HD], bf, kind="ExternalInput")
    wc = nc.dram_tensor("wc", [H_LOC * HD, D], bf, kind="ExternalInput")
    bqd = nc.dram_tensor("bq", [HD, H_LOC], f32, kind="ExternalInput")
    bkd = nc.dram_tensor("bk", [HD, KV_LOC], f32, kind="ExternalInput")
    mkd = nc.dram_tensor("mk", [128, 4 * 512], bf, kind="ExternalInput")
    out = nc.dram_tensor("out", [S, D], f32, kind="ExternalOutput")

    ACT = mybir.ActivationFunctionType

    with tile.TileContext(nc) as tc, ExitStack() as ctx:
        const = ctx.enter_context(tc.tile_pool(name="const", bufs=1))
        qkv = ctx.enter_context(tc.tile_pool(name="qkv", bufs=1))

        ones_t = const.tile([128, 128], bf, name="ones_t")
        nc.vector.memset(ones_t, 1.0)
        mask_t = const.tile([128, 4 * 512], bf, name="mask_t")
        nc.sync.dma_start(mask_t, mkd[:, :])
        bq_t = const.tile([128, H_LOC], f32, name="bq_t")
        nc.sync.dma_start(bq_t, bqd[:, :])
        bk_t = const.tile([128, KV_LOC], f32, name="bk_t")
        nc.sync.dma_start(bk_t, bkd[:, :])

        qT = {}   # (h, t) -> [128, 512]
        kT = {}   # (g, jt) -> [128, 512]
        vv = {}   # (g, jb) -> [128, 128]
        oT = {}   # (h, t) -> [128, 512]

        # ---------------- stage A: projections ----------------
        with tc.tile_pool(name="xa", bufs=1) as xa, \
             tc.tile_pool(name="wqa", bufs=2) as wqa, \
             tc.tile_pool(name="pa", bufs=2, space="PSUM") as pa:

            xTt = {}
            for t in range(NT):
                for c in range(NCH):
                    xt_ = xa.tile([128, 512], bf, name=f"x{c}_{t}", tag=f"x{c}_{t}")
                    nc.sync.dma_start(xt_, xT[c * 128:(c + 1) * 128,
                                               t * 512:(t + 1) * 512])
                    xTt[(c, t)] = xt_

            wvt = []
            for c in range(NCH):
                wv_ = xa.tile([128, KV_LOC * HD], bf, name=f"wv{c}", tag=f"wv{c}")
                nc.sync.dma_start(wv_, wv[c * 128:(c + 1) * 128, :])
                wvt.append(wv_)

            # K projection: kT[g] = Wk_g.T @ x.T  (+ bk at eviction)
            for g in range(KV_LOC):
                wkt = []
                for c in range(NCH):
                    wk_ = xa.tile([128, HD], bf, name=f"wk{g}_{c}", tag=f"wk{g}_{c}")
                    nc.sync.dma_start(wk_, wk[c * 128:(c + 1) * 128,
                                              g * HD:(g + 1) * HD])
                    wkt.append(wk_)
                for jt in range(NT):
                    psk = pa.tile([128, 512], mybir.dt.float32, name="psk", tag="pq")
                    for c in range(NCH):
                        nc.tensor.matmul(psk, lhsT=wkt[c], rhs=xTt[(c, jt)],
                                         start=(c == 0), stop=(c == NCH - 1))
                    kt_ = qkv.tile([128, 512], bf, name=f"k{g}_{jt}", tag=f"k{g}_{jt}")
                    nc.scalar.activation(kt_, psk, ACT.Identity,
                                         bias=bk_t[:, g:g + 1], scale=1.0)
                    kT[(g, jt)] = kt_

            # V projection: v[j,d] natural; lhsT = xT chunk, rhs = Wv chunk
            for jb in range(NJB):
                psv = pa.tile([128, KV_LOC * HD], mybir.dt.float32,
                              name="psv", tag="pv")
                jt, jr = jb // 4, jb % 4
                for c in range(NCH):
                    nc.tensor.matmul(psv, lhsT=xTt[(c, jt)][:, jr * 128:(jr + 1) * 128],
                                     rhs=wvt[c], start=(c == 0), stop=(c == NCH - 1))
                for g in range(KV_LOC):
                    v_ = qkv.tile([128, HD], bf, name=f"v{g}_{jb}", tag=f"v{g}_{jb}")
                    nc.vector.tensor_copy(v_, psv[:, g * HD:(g + 1) * HD])
                    vv[(g, jb)] = v_

            # Q projection: qT[h] = Wq_h.T @ x.T  (+ bq at eviction)
            for h in range(H_LOC):
                wqt = []
                for c in range(NCH):
                    wq_ = wqa.tile([128, HD], bf, name=f"wq{c}", tag=f"wq{c}")
                    nc.sync.dma_start(wq_, wq[c * 128:(c + 1) * 128,
                                              h * HD:(h + 1) * HD])
                    wqt.append(wq_)
                for t in range(NT):
                    psq = pa.tile([128, 512], mybir.dt.float32, name="psq", tag="pq")
                    for c in range(NCH):
                        nc.tensor.matmul(psq, lhsT=wqt[c], rhs=xTt[(c, t)],
                                         start=(c == 0), stop=(c == NCH - 1))
                    qt_ = qkv.tile([128, 512], bf, name=f"q{h}_{t}", tag=f"q{h}_{t}")
                    nc.scalar.activation(qt_, psq, ACT.Identity,
                                         bias=bq_t[:, h:h + 1], scale=1.0)
                    qT[(h, t)] = qt_

        # ---------------- stage B/C: attention + c_proj ----------------
        with tc.tile_pool(name="wcp", bufs=1) as wcp, \
             tc.tile_pool(name="etp", bufs=3) as etp, \
             tc.tile_pool(name="recp", bufs=2) as recp, \
             tc.tile_pool(name="outp", bufs=3) as outp, \
             tc.tile_pool(name="pb", bufs=2, space="PSUM") as pb:

            wct = {}
            for h in range(H_LOC):
                for dn in range(4):
                    wc_ = wcp.tile([128, 512], bf, name=f"wc{h}_{dn}",
                                   tag=f"wc{h}_{dn}")
                    nc.sync.dma_start(wc_, wc[h * HD:(h + 1) * HD,
                                              dn * 512:(dn + 1) * 512])
                    wct[(h, dn)] = wc_

            for t in range(NT):
                for h in range(H_LOC):
                    g = h // GROUP
                    njb = 4 * t + 4
                    pso = pb.tile([128, 512], mybir.dt.float32, name="pso", tag="o")
                    psl = pb.tile([128, 512], mybir.dt.float32, name="psl", tag="l")

                    # scores for jb, software-pipelined one block ahead
                    pss_tiles = []
                    def emit_scores(jb):
                        pss = pb.tile([128, 512], mybir.dt.float32,
                                      name="pss", tag="s")
                        jt, jr = jb // 4, jb % 4
                        nc.tensor.matmul(pss,
                                         lhsT=kT[(g, jt)][:, jr * 128:(jr + 1) * 128],
                                         rhs=qT[(h, t)], start=True, stop=True)
                        pss_tiles.append(pss)

                    emit_scores(0)
                    for jb in range(njb):
                        if jb + 1 < njb:
                            emit_scores(jb + 1)
                        pss = pss_tiles[jb]
                        et = etp.tile([128, 512], bf, name="et", tag="et")
                        nc.scalar.activation(et, pss, ACT.Exp, scale=SCALE)
                        r = jb - 4 * t
                        if r >= 0:  # diagonal band: causal 0/1 mask
                            nc.vector.tensor_mul(et, et,
                                                 mask_t[:, r * 512:(r + 1) * 512])
                        nc.tensor.matmul(pso, lhsT=vv[(g, jb)], rhs=et,
                                         start=(jb == 0), stop=(jb == njb - 1))
                        nc.tensor.matmul(psl, lhsT=ones_t, rhs=et,
                                         start=(jb == 0), stop=(jb == njb - 1))

                    rec = recp.tile([128, 512], mybir.dt.float32, name="rec",
                                    tag="rec")
                    nc.vector.reciprocal(rec, psl)
                    ot_ = qkv.tile([128, 512], bf, name=f"o{h}_{t}", tag=f"o{h}_{t}")
                    nc.vector.tensor_mul(ot_, pso, rec)
                    oT[(h, t)] = ot_

                # c_proj for this t's four 128-row i-blocks
                for ib in range(4):
                    i0 = t * 512 + ib * 128
                    for dn in range(4):
                        psc = pb.tile([128, 512], mybir.dt.float32,
                                      name="psc", tag="c")
                        for h in range(H_LOC):
                            nc.tensor.matmul(psc,
                                             lhsT=oT[(h, t)][:, ib * 128:(ib + 1) * 128],
                                             rhs=wct[(h, dn)],
                                             start=(h == 0), stop=(h == H_LOC - 1))
                        ob = outp.tile([128, 512], mybir.dt.float32, name="ob",
                                       tag="ob")
                        nc.vector.tensor_copy(ob, psc)
                        nc.sync.dma_start(out[i0:i0 + 128, dn * 512:(dn + 1) * 512],
                                          ob)
    nc.finalize()
    return nc


_NC_CACHE = None


def _get_program():
    global _NC_CACHE
    if _NC_CACHE is None:
        _NC_CACHE = _build_program()
    return _NC_CACHE


def _causal_masks():
    # mask_r[p, f] = 1 if f >= 128*r + p else 0, r = jb - 4*t in 0..3
    p = np.arange(128)[:, None]
    f = np.arange(512)[None, :]
    parts = [(f >= 128 * r + p).astype(BF16) for r in range(4)]
    return np.concatenate(parts, axis=1)  # [128, 2048]


def kernel(x, Wq, bq, Wkv, bkv, Wc, bc, _trace=False):
    global LAST_RESULTS
    x = np.asarray(x, np.float32)
    Wq = np.asarray(Wq, np.float32)
    bq = np.asarray(bq, np.float32)
    Wkv = np.asarray(Wkv, np.float32)
    bkv = np.asarray(bkv, np.float32)
    Wc = np.asarray(Wc, np.float32)
    bc = np.asarray(bc, np.float32)

    kvd = KV_LOC * HD  # 256
    mask = _causal_masks()

    # per-head-group shards (shared across the 4 batches)
    shard = []
    for hg in range(2):
        wq_s = np.ascontiguousarray(Wq[:, hg * 1024:(hg + 1) * 1024]).astype(BF16)
        wk_s = np.ascontiguousarray(Wkv[:, hg * kvd:(hg + 1) * kvd]).astype(BF16)
        wv_s = np.ascontiguousarray(
            Wkv[:, 512 + hg * kvd:512 + (hg + 1) * kvd]).astype(BF16)
        wc_s = np.ascontiguousarray(Wc[hg * 1024:(hg + 1) * 1024, :]).astype(BF16)
        bq_s = np.ascontiguousarray(
            bq[hg * 1024:(hg + 1) * 1024].reshape(H_LOC, HD).T).astype(np.float32)
        bk_s = np.ascontiguousarray(
            bkv[hg * kvd:(hg + 1) * kvd].reshape(KV_LOC, HD).T).astype(np.float32)
        shard.append((wq_s, wk_s, wv_s, wc_s, bq_s, bk_s))

    xT_b = [np.ascontiguousarray(x[b].T).astype(BF16) for b in range(B)]

    in_maps = []
    for core in range(8):
        b, hg = core // 2, core % 2
        wq_s, wk_s, wv_s, wc_s, bq_s, bk_s = shard[hg]
        in_maps.append({
            "xt": xT_b[b], "wq": wq_s, "wk": wk_s, "wv": wv_s, "wc": wc_s,
            "bq": bq_s, "bk": bk_s, "mk": mask,
        })

    nc = _get_program()
    res = bass_utils.run_bass_kernel_spmd(
        nc, in_maps, core_ids=list(range(8)), trace=_trace)
    LAST_RESULTS = res

    # bias corrections done on host: v-bias propagates as a constant row
    # through attention (rows of P sum to 1), then through c_proj; bc direct.
    u_full = np.concatenate(
        [bkv[512 + (h // GROUP) * HD:512 + (h // GROUP + 1) * HD]
         for h in range(N_HEAD)])
    corr = (u_full @ Wc + bc).astype(np.float32)  # [2048]

    out = np.empty((B, S, D), np.float32)
    for b in range(B):
        out[b] = res.results[2 * b]["out"] + res.results[2 * b + 1]["out"] + corr
    return out
